# revision 1
# baseline (speedup 1.0000x reference)
"""nn_CustomLSTM kernel for 8 trn2 NeuronCores.

Strategy: hidden/gate-sharded recurrence. Core c owns hidden units
[128c:128c+128) and their 4 gate rows (512 rows of the 4H=4096 gate matrix,
host-packed as [f|i|c~|o] blocks of 128). Per timestep each core computes its
gate slice with full-batch matmuls (formulation: out[batch=64, 512] =
hT_tile.T @ WhT_tile accumulated over 8 k-tiles), updates its c/h slice, and
all-gathers the transposed h slices so every core has h^T for the next step.
x projections (x_t @ Wx^T + b) are precomputed on-device into DRAM in one
efficient pass. Final FC runs on every core redundantly; core 0's output is
returned.

B=64, T=512, I=512, H=1024, S=512."""

import sys

if "/opt/trn_rl_repo" not in sys.path:
    sys.path.insert(0, "/opt/trn_rl_repo")

import numpy as np

import concourse.bass as bass
import concourse.mybir as mybir
import concourse.tile as tile
from concourse import bass_utils
from concourse.masks import make_identity
from concourse.vector_clock import ScopedClock

F32 = mybir.dt.float32
AF = mybir.ActivationFunctionType

B, T, I, H, S = 64, 512, 512, 1024, 512
NC = 8
HSL = H // NC  # 128 hidden units per core
GSL = 4 * HSL  # 512 gate rows per core
KT_H = H // 128  # 8 h k-tiles
KT_I = I // 128  # 4 x k-tiles

# matmul input dtype for the recurrent part: float32 (safe) or float16 (2.8x
# faster streaming, ~1e-3 per-step rounding).
MM_DT = mybir.dt.float16

# ---------------------------------------------------------------- patches


def _patch_tile_drain():
    """Walrus here rejects >1 sync wait on Drain; split across SP nops."""

    def _drain_and_barrier(self, tick_clock, wait_clock):
        nop_inst = self.nc.sync.nop(nofuse=True)
        wait_clock.add_sem_waits(
            nop_inst.ins, ScopedClock({None: tick_clock.global_clock})
        )
        waits = list(nop_inst.ins.sync_info.on_wait)
        if len(waits) > 1:
            nop_inst.ins.sync_info.on_wait = waits[:1]
            for w in waits[1:]:
                extra = self.nc.sync.nop(nofuse=True)
                extra.ins.sync_info = mybir.SyncInfo(on_wait=[w], on_update=[])
        self.nc.sync.drain()
        self.nc.all_engine_barrier()
        assert self.sems is not None
        popped = self.nc._tile_sem_poison_stack.pop()
        assert popped is self._sem_poison
        self.nc.clear_and_free_semaphores(list(self.sems.allocated().values()))
        self.nc.all_engine_barrier()

    tile.TileContext._drain_and_barrier = _drain_and_barrier


_patch_tile_drain()


def _hoist_excess_waits(nc, max_waits=1):
    """Walrus rejects >1 sync wait on several instruction structs; hoist
    excess waits onto same-engine nops (engine queues issue in order)."""
    for f in nc.m.functions:
        for b in f.blocks:
            out = []
            for inst in b.instructions:
                si = inst.sync_info
                if si is not None and len(si.on_wait) > max_waits:
                    waits = list(si.on_wait)
                    excess = waits[: len(waits) - max_waits]
                    si.on_wait = waits[len(waits) - max_waits :]
                    for i in range(0, len(excess), max_waits):
                        out.append(
                            mybir.InstNoOp(
                                name=nc.get_next_instruction_name(),
                                engine=inst.engine,
                                bass_nofuse=True,
                                sync_info=mybir.SyncInfo(
                                    on_wait=excess[i : i + max_waits], on_update=[]
                                ),
                            )
                        )
                out.append(inst)
            b.instructions = out


# ---------------------------------------------------------------- builder


def build_nc(steps=T, hoist=True):
    nc = bass.Bass(num_devices=NC)
    XT = nc.declare_dram_parameter("XT", [I, B * steps], F32, isOutput=False)
    WxT = nc.declare_dram_parameter("WxT", [KT_I, 128, GSL], F32, isOutput=False)
    WhT = nc.declare_dram_parameter("WhT", [KT_H, 128, GSL], F32, isOutput=False)
    b_rep = nc.declare_dram_parameter("b_rep", [128, GSL], F32, isOutput=False)
    fcwT = nc.declare_dram_parameter("fcwT", [KT_H, 128, S], F32, isOutput=False)
    fcb = nc.declare_dram_parameter("fcb", [B, S], F32, isOutput=False)
    y = nc.declare_dram_parameter("y", [B, S], F32, isOutput=True)

    n_mrows = B * steps // 128  # xproj m-tiles

    with tile.TileContext(nc) as tc:
        with (
            tc.tile_pool(name="const", bufs=1) as cpool,
            tc.tile_pool(name="xw", bufs=3) as xw,
            tc.tile_pool(name="xp", bufs=4) as xpp,
            tc.tile_pool(name="work", bufs=3) as wk,
            tc.tile_pool(name="hbuf", bufs=3) as hb,
            tc.tile_pool(name="ps", bufs=2, space="PSUM") as ps,
            tc.tile_pool(name="pst", bufs=2, space="PSUM") as pst,
            tc.tile_pool(name="dram", bufs=1, space="DRAM") as dram,
            tc.tile_pool(name="agd", bufs=2, space="DRAM") as agd,
        ):
            ident = cpool.tile([128, 128], F32, tag="ident")
            make_identity(nc, ident[:, :])
            WhT_sb = cpool.tile([128, KT_H * GSL], MM_DT, tag="WhT")
            WxT_sb = cpool.tile([128, KT_I * GSL], F32, tag="WxT")
            b_sb = cpool.tile([128, GSL], F32, tag="b")
            fcw_sb = cpool.tile([128, KT_H * S], F32, tag="fcw")
            fcb_sb = cpool.tile([B, S], F32, tag="fcb")
            if MM_DT != F32:
                WhT_f32 = cpool.tile([128, KT_H * GSL], F32, tag="WhTf32")
                for k in range(KT_H):
                    nc.sync.dma_start(
                        out=WhT_f32[:, GSL * k : GSL * (k + 1)], in_=WhT[k, :, :]
                    )
                nc.vector.tensor_copy(WhT_sb[:, :], WhT_f32[:, :])
            else:
                for k in range(KT_H):
                    nc.sync.dma_start(
                        out=WhT_sb[:, GSL * k : GSL * (k + 1)], in_=WhT[k, :, :]
                    )
            for k in range(KT_I):
                nc.sync.dma_start(
                    out=WxT_sb[:, GSL * k : GSL * (k + 1)], in_=WxT[k, :, :]
                )
            nc.sync.dma_start(out=b_sb[:, :], in_=b_rep[:, :])
            for k in range(KT_H):
                nc.sync.dma_start(
                    out=fcw_sb[:, S * k : S * (k + 1)], in_=fcwT[k, :, :]
                )
            nc.sync.dma_start(out=fcb_sb[:, :], in_=fcb[:, :])

            xp_dram = dram.tile([B * steps, GSL], F32, tag="xp")

            # ---- phase 1: x projections (+bias) for all timesteps ----
            for m in range(n_mrows):
                xt_sb = xw.tile([128, KT_I * 128], F32, tag="xt")
                for k in range(KT_I):
                    nc.sync.dma_start(
                        out=xt_sb[:, 128 * k : 128 * (k + 1)],
                        in_=XT[128 * k : 128 * (k + 1), 128 * m : 128 * (m + 1)],
                    )
                psx = ps.tile([128, GSL], F32, tag="psx")
                for k in range(KT_I):
                    nc.tensor.matmul(
                        psx[:, :],
                        lhsT=xt_sb[:, 128 * k : 128 * (k + 1)],
                        rhs=WxT_sb[:, GSL * k : GSL * (k + 1)],
                        start=(k == 0),
                        stop=(k == KT_I - 1),
                    )
                xpm = xw.tile([128, GSL], F32, tag="xpm")
                nc.vector.tensor_add(xpm[:, :], psx[:, :], b_sb[:, :])
                nc.sync.dma_start(
                    out=xp_dram[128 * m : 128 * (m + 1), :], in_=xpm[:, :]
                )

            # ---- phase 2: recurrence ----
            c_cur = None
            hT_cur = None  # [128, NC*64] sbuf tile: k-tile c at cols 64c
            for t in range(steps):
                xpt = xpp.tile([B, GSL], F32, tag="xpt")
                nc.sync.dma_start(out=xpt[:, :], in_=xp_dram[B * t : B * (t + 1), :])
                gates = wk.tile([B, GSL], F32, tag="gates")
                if t == 0:
                    nc.vector.tensor_copy(gates[:, :], xpt[:, :])
                else:
                    psg = ps.tile([B, GSL], F32, tag="psg")
                    for k in range(KT_H):
                        nc.tensor.matmul(
                            psg[:, :],
                            lhsT=hT_cur[:, 64 * k : 64 * (k + 1)],
                            rhs=WhT_sb[:, GSL * k : GSL * (k + 1)],
                            start=(k == 0),
                            stop=(k == KT_H - 1),
                        )
                    nc.vector.tensor_add(gates[:, :], psg[:, :], xpt[:, :])
                acts = wk.tile([B, GSL], F32, tag="acts")
                nc.scalar.activation(acts[:, 0:256], gates[:, 0:256], AF.Sigmoid)
                nc.scalar.activation(acts[:, 256:384], gates[:, 256:384], AF.Tanh)
                nc.scalar.activation(acts[:, 384:512], gates[:, 384:512], AF.Sigmoid)
                c_new = wk.tile([B, HSL], F32, tag="c")
                fc_t = wk.tile([B, HSL], F32, tag="fc")
                ic_t = wk.tile([B, HSL], F32, tag="ic")
                if t == 0:
                    nc.vector.tensor_mul(
                        c_new[:, :], acts[:, 128:256], acts[:, 256:384]
                    )
                else:
                    nc.vector.tensor_mul(fc_t[:, :], acts[:, 0:128], c_cur[:, :])
                    nc.vector.tensor_mul(
                        ic_t[:, :], acts[:, 128:256], acts[:, 256:384]
                    )
                    nc.vector.tensor_add(c_new[:, :], fc_t[:, :], ic_t[:, :])
                c_cur = c_new
                tnh = wk.tile([B, HSL], F32, tag="tnh")
                nc.scalar.activation(tnh[:, :], c_new[:, :], AF.Tanh)
                h_new = wk.tile([B, HSL], F32, tag="h")
                nc.vector.tensor_mul(h_new[:, :], acts[:, 384:512], tnh[:, :])
                # transpose my h slice -> [128, 64]
                pstr = pst.tile([128, B], F32, tag="pstr")
                nc.tensor.transpose(pstr[:, :], h_new[:, :], ident[0:B, 0:B])
                hsl = hb.tile([128, B], MM_DT, tag="hsl")
                nc.vector.tensor_copy(hsl[:, :], pstr[:, :])
                # exchange: allgather slices -> hT_next
                ag_in = agd.tile([128, B], MM_DT, tag="agi")
                ag_out = agd.tile([NC * 128, B], MM_DT, tag="ago")
                nc.sync.dma_start(out=ag_in[:, :], in_=hsl[:, :])
                nc.gpsimd.collective_compute(
                    "AllGather",
                    mybir.AluOpType.bypass,
                    replica_groups=[list(range(NC))],
                    ins=[ag_in[:, :].opt()],
                    outs=[ag_out[:, :].opt()],
                )
                hT_next = hb.tile([128, NC * B], MM_DT, tag="hT")
                for k in range(NC):
                    nc.sync.dma_start(
                        out=hT_next[:, B * k : B * (k + 1)],
                        in_=ag_out[128 * k : 128 * (k + 1), :],
                    )
                hT_cur = hT_next

            # ---- final FC on full hT (redundant on every core) ----
            hT_f32 = wk.tile([128, NC * B], F32, tag="hTf")
            if MM_DT != F32:
                nc.vector.tensor_copy(hT_f32[:, :], hT_cur[:, :])
            else:
                hT_f32 = hT_cur
            psy = ps.tile([B, S], F32, tag="psy")
            for k in range(KT_H):
                nc.tensor.matmul(
                    psy[:, :],
                    lhsT=hT_f32[:, 64 * k : 64 * (k + 1)],
                    rhs=fcw_sb[:, S * k : S * (k + 1)],
                    start=(k == 0),
                    stop=(k == KT_H - 1),
                )
            y_sb = wk.tile([B, S], F32, tag="y")
            nc.vector.tensor_add(y_sb[:, :], psy[:, :], fcb_sb[:, :])
            nc.sync.dma_start(out=y[:, :], in_=y_sb[:, :])

    if hoist:
        _hoist_excess_waits(nc)
    return nc


# ---------------------------------------------------------------- host


def prep_inputs(x, W_f, b_f, W_i, b_i, W_c, b_c, W_o, b_o, fc_w, fc_b, steps=T):
    """Build the 8 per-core input maps (all numpy, no device work)."""
    x = x[:, :steps, :]
    # X^T with flat row order (t, b)
    XT = np.ascontiguousarray(
        x.transpose(1, 0, 2).reshape(steps * B, I).T
    )  # [I, steps*B]
    fcwT = np.ascontiguousarray(fc_w.T).reshape(KT_H, 128, S)
    fcb_rep = np.broadcast_to(fc_b, (B, S)).copy()
    in_maps = []
    for c in range(NC):
        hs = slice(HSL * c, HSL * (c + 1))
        Wsl = np.concatenate(
            [W_f[hs], W_i[hs], W_c[hs], W_o[hs]], axis=0
        )  # [512, H+I]
        WhT = np.ascontiguousarray(Wsl[:, :H].T).reshape(KT_H, 128, GSL)
        WxT = np.ascontiguousarray(Wsl[:, H:].T).reshape(KT_I, 128, GSL)
        bsl = np.concatenate([b_f[hs], b_i[hs], b_c[hs], b_o[hs]])  # [512]
        b_rep = np.broadcast_to(bsl, (128, GSL)).copy()
        in_maps.append(
            {
                "XT": XT,
                "WxT": WxT.astype(np.float32),
                "WhT": WhT.astype(np.float32),
                "b_rep": b_rep.astype(np.float32),
                "fcwT": fcwT.astype(np.float32),
                "fcb": fcb_rep.astype(np.float32),
            }
        )
    return in_maps


_CACHED = {}


def kernel(**inputs) -> np.ndarray:
    steps = inputs["x"].shape[1]
    if steps not in _CACHED:
        _CACHED[steps] = build_nc(steps)
    nc = _CACHED[steps]
    in_maps = prep_inputs(**inputs, steps=steps)
    res = bass_utils.run_bass_kernel_spmd(
        nc, in_maps, core_ids=list(range(NC)), trace=False
    )
    return res.results[0]["y"]


if __name__ == "__main__":
    rng = np.random.default_rng(0)
    stdv = 1.0 / np.sqrt(H)
    u = lambda *s: rng.uniform(-stdv, stdv, s).astype(np.float32)
    inputs = dict(
        x=rng.standard_normal((B, T, I), dtype=np.float32),
        W_f=u(H, H + I), b_f=u(H), W_i=u(H, H + I), b_i=u(H),
        W_c=u(H, H + I), b_c=u(H), W_o=u(H, H + I), b_o=u(H),
        fc_w=u(S, H), fc_b=u(S),
    )
    out = kernel(**inputs)
    print("out", out.shape, out.dtype)



# revision 4
# speedup vs baseline: 1.0542x; 1.0542x over previous
"""nn_CustomLSTM kernel for 8 trn2 NeuronCores.

Strategy: hidden/gate-sharded recurrence. Core c owns hidden units
[128c:128c+128) and their 4 gate rows (512 rows of the 4H=4096 gate matrix,
host-packed as [f|i|c~|o] blocks of 128). Per timestep each core computes its
gate slice with full-batch matmuls (formulation: out[batch=64, 512] =
hT_tile.T @ WhT_tile accumulated over 8 k-tiles), updates its c/h slice, and
all-gathers the transposed h slices so every core has h^T for the next step.
x projections (x_t @ Wx^T + b) are precomputed on-device into DRAM in one
efficient pass. Final FC runs on every core redundantly; core 0's output is
returned.

B=64, T=512, I=512, H=1024, S=512."""

import sys

if "/opt/trn_rl_repo" not in sys.path:
    sys.path.insert(0, "/opt/trn_rl_repo")

import numpy as np

import concourse.bass as bass
import concourse.mybir as mybir
import concourse.tile as tile
from concourse import bass_utils
from concourse.masks import make_identity
from concourse.vector_clock import ScopedClock

F32 = mybir.dt.float32
AF = mybir.ActivationFunctionType

B, T, I, H, S = 64, 512, 512, 1024, 512
NC = 8
HSL = H // NC  # 128 hidden units per core
GSL = 4 * HSL  # 512 gate rows per core
KT_H = H // 128  # 8 h k-tiles
KT_I = I // 128  # 4 x k-tiles

# matmul input dtype for the recurrent part: float32 (safe) or float16 (2.8x
# faster streaming, ~1e-3 per-step rounding).
MM_DT = mybir.dt.float16

# ---------------------------------------------------------------- patches


def _patch_tile_drain():
    """Walrus here rejects >1 sync wait on Drain; split across SP nops."""

    def _drain_and_barrier(self, tick_clock, wait_clock):
        nop_inst = self.nc.sync.nop(nofuse=True)
        wait_clock.add_sem_waits(
            nop_inst.ins, ScopedClock({None: tick_clock.global_clock})
        )
        waits = list(nop_inst.ins.sync_info.on_wait)
        if len(waits) > 1:
            nop_inst.ins.sync_info.on_wait = waits[:1]
            for w in waits[1:]:
                extra = self.nc.sync.nop(nofuse=True)
                extra.ins.sync_info = mybir.SyncInfo(on_wait=[w], on_update=[])
        self.nc.sync.drain()
        self.nc.all_engine_barrier()
        assert self.sems is not None
        popped = self.nc._tile_sem_poison_stack.pop()
        assert popped is self._sem_poison
        self.nc.clear_and_free_semaphores(list(self.sems.allocated().values()))
        self.nc.all_engine_barrier()

    tile.TileContext._drain_and_barrier = _drain_and_barrier


_patch_tile_drain()


def _hoist_excess_waits(nc, max_waits=1):
    """Walrus rejects >1 sync wait on several instruction structs; hoist
    excess waits onto same-engine nops (engine queues issue in order)."""
    for f in nc.m.functions:
        for b in f.blocks:
            out = []
            for inst in b.instructions:
                si = inst.sync_info
                if si is not None and len(si.on_wait) > max_waits:
                    waits = list(si.on_wait)
                    excess = waits[: len(waits) - max_waits]
                    si.on_wait = waits[len(waits) - max_waits :]
                    for i in range(0, len(excess), max_waits):
                        out.append(
                            mybir.InstNoOp(
                                name=nc.get_next_instruction_name(),
                                engine=inst.engine,
                                bass_nofuse=True,
                                sync_info=mybir.SyncInfo(
                                    on_wait=excess[i : i + max_waits], on_update=[]
                                ),
                            )
                        )
                out.append(inst)
            b.instructions = out


# ---------------------------------------------------------------- builder


def build_nc(steps=T, hoist=True):
    nc = bass.Bass(num_devices=NC)
    XT = nc.declare_dram_parameter("XT", [I, B * steps], F32, isOutput=False)
    WxT = nc.declare_dram_parameter("WxT", [KT_I, 128, GSL], F32, isOutput=False)
    WhT = nc.declare_dram_parameter("WhT", [KT_H, 128, GSL], F32, isOutput=False)
    b_rep = nc.declare_dram_parameter("b_rep", [128, GSL], F32, isOutput=False)
    fcwT = nc.declare_dram_parameter("fcwT", [KT_H, 128, S], F32, isOutput=False)
    fcb = nc.declare_dram_parameter("fcb", [B, S], F32, isOutput=False)
    y = nc.declare_dram_parameter("y", [B, S], F32, isOutput=True)

    n_mrows = B * steps // 128  # xproj m-tiles

    with tile.TileContext(nc) as tc:
        with (
            tc.tile_pool(name="const", bufs=1) as cpool,
            tc.tile_pool(name="xw", bufs=3) as xw,
            tc.tile_pool(name="xp", bufs=4) as xpp,
            tc.tile_pool(name="work", bufs=3) as wk,
            tc.tile_pool(name="hbuf", bufs=3) as hb,
            tc.tile_pool(name="ps", bufs=2, space="PSUM") as ps,
            tc.tile_pool(name="pst", bufs=2, space="PSUM") as pst,
            tc.tile_pool(name="dram", bufs=1, space="DRAM") as dram,
            tc.tile_pool(name="agd", bufs=2, space="DRAM") as agd,
        ):
            ident = cpool.tile([128, 128], F32, tag="ident")
            make_identity(nc, ident[:, :])
            WhT_sb = cpool.tile([128, KT_H * GSL], MM_DT, tag="WhT")
            WxT_sb = cpool.tile([128, KT_I * GSL], F32, tag="WxT")
            b_sb = cpool.tile([128, GSL], F32, tag="b")
            fcw_sb = cpool.tile([128, KT_H * S], F32, tag="fcw")
            fcb_sb = cpool.tile([B, S], F32, tag="fcb")
            if MM_DT != F32:
                WhT_f32 = cpool.tile([128, KT_H * GSL], F32, tag="WhTf32")
                for k in range(KT_H):
                    nc.sync.dma_start(
                        out=WhT_f32[:, GSL * k : GSL * (k + 1)], in_=WhT[k, :, :]
                    )
                nc.vector.tensor_copy(WhT_sb[:, :], WhT_f32[:, :])
            else:
                for k in range(KT_H):
                    nc.sync.dma_start(
                        out=WhT_sb[:, GSL * k : GSL * (k + 1)], in_=WhT[k, :, :]
                    )
            for k in range(KT_I):
                nc.sync.dma_start(
                    out=WxT_sb[:, GSL * k : GSL * (k + 1)], in_=WxT[k, :, :]
                )
            nc.sync.dma_start(out=b_sb[:, :], in_=b_rep[:, :])
            for k in range(KT_H):
                nc.sync.dma_start(
                    out=fcw_sb[:, S * k : S * (k + 1)], in_=fcwT[k, :, :]
                )
            nc.sync.dma_start(out=fcb_sb[:, :], in_=fcb[:, :])

            xp_dram = dram.tile([B * steps, GSL], F32, tag="xp")

            # ---- phase 1: x projections (+bias) for all timesteps ----
            for m in range(n_mrows):
                xt_sb = xw.tile([128, KT_I * 128], F32, tag="xt")
                for k in range(KT_I):
                    nc.sync.dma_start(
                        out=xt_sb[:, 128 * k : 128 * (k + 1)],
                        in_=XT[128 * k : 128 * (k + 1), 128 * m : 128 * (m + 1)],
                    )
                psx = ps.tile([128, GSL], F32, tag="psx")
                for k in range(KT_I):
                    nc.tensor.matmul(
                        psx[:, :],
                        lhsT=xt_sb[:, 128 * k : 128 * (k + 1)],
                        rhs=WxT_sb[:, GSL * k : GSL * (k + 1)],
                        start=(k == 0),
                        stop=(k == KT_I - 1),
                    )
                xpm = xw.tile([128, GSL], F32, tag="xpm")
                nc.vector.tensor_add(xpm[:, :], psx[:, :], b_sb[:, :])
                nc.sync.dma_start(
                    out=xp_dram[128 * m : 128 * (m + 1), :], in_=xpm[:, :]
                )

            # ---- phase 2: recurrence ----
            c_cur = None
            hT_cur = None  # [128, NC*64] sbuf tile: k-tile c at cols 64c
            for t in range(steps):
                xpt = xpp.tile([B, GSL], F32, tag="xpt")
                nc.sync.dma_start(out=xpt[:, :], in_=xp_dram[B * t : B * (t + 1), :])
                gates = wk.tile([B, GSL], F32, tag="gates")
                if t == 0:
                    nc.vector.tensor_copy(gates[:, :], xpt[:, :])
                else:
                    psg = ps.tile([B, GSL], F32, tag="psg")
                    for k in range(KT_H):
                        nc.tensor.matmul(
                            psg[:, :],
                            lhsT=hT_cur[:, 64 * k : 64 * (k + 1)],
                            rhs=WhT_sb[:, GSL * k : GSL * (k + 1)],
                            start=(k == 0),
                            stop=(k == KT_H - 1),
                        )
                    nc.vector.tensor_add(gates[:, :], psg[:, :], xpt[:, :])
                acts = wk.tile([B, GSL], F32, tag="acts")
                nc.scalar.activation(acts[:, 0:256], gates[:, 0:256], AF.Sigmoid)
                nc.scalar.activation(acts[:, 256:384], gates[:, 256:384], AF.Tanh)
                nc.scalar.activation(acts[:, 384:512], gates[:, 384:512], AF.Sigmoid)
                c_new = wk.tile([B, HSL], F32, tag="c")
                fc_t = wk.tile([B, HSL], F32, tag="fc")
                ic_t = wk.tile([B, HSL], F32, tag="ic")
                if t == 0:
                    nc.vector.tensor_mul(
                        c_new[:, :], acts[:, 128:256], acts[:, 256:384]
                    )
                else:
                    nc.vector.tensor_mul(fc_t[:, :], acts[:, 0:128], c_cur[:, :])
                    nc.vector.tensor_mul(
                        ic_t[:, :], acts[:, 128:256], acts[:, 256:384]
                    )
                    nc.vector.tensor_add(c_new[:, :], fc_t[:, :], ic_t[:, :])
                c_cur = c_new
                tnh = wk.tile([B, HSL], F32, tag="tnh")
                nc.scalar.activation(tnh[:, :], c_new[:, :], AF.Tanh)
                h_new = wk.tile([B, HSL], F32, tag="h")
                nc.vector.tensor_mul(h_new[:, :], acts[:, 384:512], tnh[:, :])
                # transpose my h slice -> [128, 64]
                pstr = pst.tile([128, B], F32, tag="pstr")
                nc.tensor.transpose(pstr[:, :], h_new[:, :], ident[0:B, 0:B])
                hsl = hb.tile([128, B], MM_DT, tag="hsl")
                nc.vector.tensor_copy(hsl[:, :], pstr[:, :])
                # exchange: allgather slices -> hT_next
                ag_in = agd.tile([128, B], MM_DT, tag="agi")
                ag_out = agd.tile([NC * 128, B], MM_DT, tag="ago")
                nc.sync.dma_start(out=ag_in[:, :], in_=hsl[:, :])
                nc.gpsimd.collective_compute(
                    "AllGather",
                    mybir.AluOpType.bypass,
                    replica_groups=[list(range(NC))],
                    ins=[ag_in[:, :].opt()],
                    outs=[ag_out[:, :].opt()],
                )
                hT_next = hb.tile([128, NC * B], MM_DT, tag="hT")
                # Alternate load queues: 8 serialized ~600ns issues on the
                # sync engine made late k-tile matmuls stall on their slice.
                for k in range(NC):
                    eng = nc.sync if k % 2 == 0 else nc.gpsimd
                    eng.dma_start(
                        out=hT_next[:, B * k : B * (k + 1)],
                        in_=ag_out[128 * k : 128 * (k + 1), :],
                    )
                hT_cur = hT_next

            # ---- final FC on full hT (redundant on every core) ----
            hT_f32 = wk.tile([128, NC * B], F32, tag="hTf")
            if MM_DT != F32:
                nc.vector.tensor_copy(hT_f32[:, :], hT_cur[:, :])
            else:
                hT_f32 = hT_cur
            psy = ps.tile([B, S], F32, tag="psy")
            for k in range(KT_H):
                nc.tensor.matmul(
                    psy[:, :],
                    lhsT=hT_f32[:, 64 * k : 64 * (k + 1)],
                    rhs=fcw_sb[:, S * k : S * (k + 1)],
                    start=(k == 0),
                    stop=(k == KT_H - 1),
                )
            y_sb = wk.tile([B, S], F32, tag="y")
            nc.vector.tensor_add(y_sb[:, :], psy[:, :], fcb_sb[:, :])
            nc.sync.dma_start(out=y[:, :], in_=y_sb[:, :])

    if hoist:
        _hoist_excess_waits(nc)
    return nc


# ---------------------------------------------------------------- host


def prep_inputs(x, W_f, b_f, W_i, b_i, W_c, b_c, W_o, b_o, fc_w, fc_b, steps=T):
    """Build the 8 per-core input maps (all numpy, no device work)."""
    x = x[:, :steps, :]
    # X^T with flat row order (t, b)
    XT = np.ascontiguousarray(
        x.transpose(1, 0, 2).reshape(steps * B, I).T
    )  # [I, steps*B]
    fcwT = np.ascontiguousarray(fc_w.T).reshape(KT_H, 128, S)
    fcb_rep = np.broadcast_to(fc_b, (B, S)).copy()
    in_maps = []
    for c in range(NC):
        hs = slice(HSL * c, HSL * (c + 1))
        Wsl = np.concatenate(
            [W_f[hs], W_i[hs], W_c[hs], W_o[hs]], axis=0
        )  # [512, H+I]
        WhT = np.ascontiguousarray(Wsl[:, :H].T).reshape(KT_H, 128, GSL)
        WxT = np.ascontiguousarray(Wsl[:, H:].T).reshape(KT_I, 128, GSL)
        bsl = np.concatenate([b_f[hs], b_i[hs], b_c[hs], b_o[hs]])  # [512]
        b_rep = np.broadcast_to(bsl, (128, GSL)).copy()
        in_maps.append(
            {
                "XT": XT,
                "WxT": WxT.astype(np.float32),
                "WhT": WhT.astype(np.float32),
                "b_rep": b_rep.astype(np.float32),
                "fcwT": fcwT.astype(np.float32),
                "fcb": fcb_rep.astype(np.float32),
            }
        )
    return in_maps


_CACHED = {}


def kernel(**inputs) -> np.ndarray:
    steps = inputs["x"].shape[1]
    if steps not in _CACHED:
        _CACHED[steps] = build_nc(steps)
    nc = _CACHED[steps]
    in_maps = prep_inputs(**inputs, steps=steps)
    res = bass_utils.run_bass_kernel_spmd(
        nc, in_maps, core_ids=list(range(NC)), trace=False
    )
    return res.results[0]["y"]


if __name__ == "__main__":
    rng = np.random.default_rng(0)
    stdv = 1.0 / np.sqrt(H)
    u = lambda *s: rng.uniform(-stdv, stdv, s).astype(np.float32)
    inputs = dict(
        x=rng.standard_normal((B, T, I), dtype=np.float32),
        W_f=u(H, H + I), b_f=u(H), W_i=u(H, H + I), b_i=u(H),
        W_c=u(H, H + I), b_c=u(H), W_o=u(H, H + I), b_o=u(H),
        fc_w=u(S, H), fc_b=u(S),
    )
    out = kernel(**inputs)
    print("out", out.shape, out.dtype)

